# revision 30
# baseline (speedup 1.0000x reference)
"""Multi-head self-attention (B=2, S=2048, D=1024, H=16, causal) on 8 TRN2 cores.

Sharding: core c handles batch b=c//4 and head-group g=c%4 (4 heads each).
Host pre-transposes/pre-tiles everything into bf16 so on-chip there are no
transposes and every DMA is one per-partition-contiguous transfer:
  xt   [4][128, 8*512]  xt[sb][p, ko*512+s] = x[b].T[ko*128+p, sb*512+s]
  wq/wk/wv [128, 8, 256] w[p,ko,m] = W.T[ko*128+p, g*256+m]
  wo   [128, 2, 1024]    wo[p,co,n] = Wo[:, g*256+co*128+p].T row
  tri  [128, 128]        tri[j,i] = (j <= i)  (diagonal-block causal mask)
Host sums the 4 per-group bf16 partial outputs per batch at the end (fp32).

Schedule (all matmuls bf16 into fp32 PSUM, built for zero PE idle so the
HAM clock gate stays at 8/8 -- any PE gap >~1-2us re-throttles it to
1.2GHz for 3.4us+):
  - startup: wq/wk/xt0 stream in ko-chunks and the sb0 q/k projections
    accumulate ko-major into two shared PSUM tiles, so the first matmul
    needs only the first weight+x chunks (~10us in, mostly fixed NEFF
    preamble); bulk loads follow on the same queue.
  - projections are interleaved with attention: q/k per Q-block between
    attention blocks, v-projections (pure PE work, psum from the psout
    pool) slid between attention chunks, so the ACT engine's exp
    (~0.9ns/elem, slightly slower per chunk than the PE's 4 matmuls)
    never rate-limits.
  - v tiles [128, 4(head), 128]: cols 0:64 hold 1.0, cols 64:128 hold v,
    so the PV matmul emits the softmax denominator replicated in PSUM
    rows 0:64 (partition 0: custom-DVE recip ignores partition offsets on
    HW) and the numerator in rows 64:128; normalization is one recip +
    one mul on DVE per head.
  - scoresT[j,i] layout: softmax needs no transpose; diagonal 512-col
    chunks are trimmed to their causal width and masked only on the
    128-wide triangle block.
  - QK(j+1) is emitted before PV(j) so exp/mask latency hides under the
    next chunk's QK matmuls; the next block's first two QK chunks are
    pre-emitted across the outproj junction.
  - outproj(Q) is emitted after attention(Q+1, mo=0) with aT split
    per-mo and so-pairs doing co=0 first, so it never waits on the mo=1
    normalization chain; osb evacuation is split ACT/DVE halves.
"""

import os
import sys

sys.path.insert(0, "/opt/trn_rl_repo")
os.environ.setdefault("MYCRO_LOCAL_CACHE", "1")

import numpy as np
import ml_dtypes

import concourse.bacc as bacc
import concourse.bass as bass
import concourse.mybir as mybir
import concourse.tile as tile
from concourse import bass_utils

# The agent image's antenv lacks axon_hooks, so bass_utils' trace path dies on
# import.  Register a shim module that lazily builds the ctypes NTFF hook.
if "antenv.axon_hooks" not in sys.modules:
    import types

    _shim = types.ModuleType("antenv.axon_hooks")
    _shim._HOOK = None

    def _set_hook(hook, _m=_shim):
        _m._HOOK = hook

    def _get_hook(_m=_shim):
        if _m._HOOK is None:
            try:
                from trn_agent_boot.trn_boot import _ntff_profile_via_ctypes

                _m._HOOK = _ntff_profile_via_ctypes("/opt/axon/libaxon_pjrt.so")
            except Exception:
                _m._HOOK = None
        return _m._HOOK

    _shim.set_axon_ntff_profile_hook = _set_hook
    _shim.get_axon_ntff_profile_hook = _get_hook
    sys.modules["antenv.axon_hooks"] = _shim

B, S, D, H = 2, 2048, 1024, 16
DK = 64                      # head dim
HC = 4                       # heads per core
GC = HC * DK                 # 256 cols per head-group
N_CORES = 8
SCALE = 1.0 / np.sqrt(DK)    # 0.125

F32 = mybir.dt.float32
BF16 = mybir.dt.bfloat16
NPBF16 = ml_dtypes.bfloat16

TRACE = False
LAST_RESULTS = None


def build_bass():
    nc = bacc.Bacc("TRN2", target_bir_lowering=False, debug=False)

    xt_d = nc.dram_tensor("xt", [4, 128, 8 * 512], BF16, kind="ExternalInput")
    wq_d = nc.dram_tensor("wq", [128, 8, GC], BF16, kind="ExternalInput")
    wk_d = nc.dram_tensor("wk", [128, 8, GC], BF16, kind="ExternalInput")
    wv_d = nc.dram_tensor("wv", [128, 8, GC], BF16, kind="ExternalInput")
    wo_d = nc.dram_tensor("wo", [128, 2, D], BF16, kind="ExternalInput")
    tri_d = nc.dram_tensor("tri", [128, 128], BF16, kind="ExternalInput")
    out_d = nc.dram_tensor("out", [S, D], BF16, kind="ExternalOutput")

    EXP = mybir.ActivationFunctionType.Exp

    with tile.TileContext(nc) as tc:
        with (
            nc.allow_low_precision(reason="bf16 matmuls, fp32 psum accumulate"),
            tc.tile_pool(name="const", bufs=1) as const,
            tc.tile_pool(name="work", bufs=4) as work,
            tc.tile_pool(name="apool", bufs=4) as apool,
            tc.tile_pool(name="opool", bufs=4) as opool,
            tc.tile_pool(name="rpool", bufs=4) as rpool,
            tc.tile_pool(name="psmm", bufs=2, space="PSUM") as psmm,
            tc.tile_pool(name="psout", bufs=4, space="PSUM") as psout,
        ):
            # ---- input DMAs, in consumption order ---------------------------
            # critical loads (wq, xt0, wk) as ~128KB pieces on the SP HWDGE
            # queue: each DMA binds one of the 16 HW rings, so small pieces
            # land early and in parallel.  Bulk goes through the gpsimd SWDGE
            # (separate descriptor-gen engine + queues) and trickles in
            # behind without delaying the criticals.
            wq = const.tile([128, 8, GC], BF16)
            wk = const.tile([128, 8, GC], BF16)
            xts = [const.tile([128, 8, 512], BF16, name=f"x{sb}")
                   for sb in range(4)]
            nc.sync.dma_start(wq[:, 0:2, :], wq_d[:, 0:2, :])
            nc.sync.dma_start(xts[0][:, 0, :], xt_d[0, :, 0:512])
            nc.sync.dma_start(xts[0][:, 1, :], xt_d[0, :, 512:1024])
            nc.sync.dma_start(wq[:, 2:4, :], wq_d[:, 2:4, :])
            nc.sync.dma_start(wk[:, 0:2, :], wk_d[:, 0:2, :])
            nc.sync.dma_start(xts[0][:, 2, :], xt_d[0, :, 1024:1536])
            nc.sync.dma_start(wq[:, 4:8, :], wq_d[:, 4:8, :])
            nc.sync.dma_start(wk[:, 2:4, :], wk_d[:, 2:4, :])
            nc.sync.dma_start(xts[0][:, 3, :], xt_d[0, :, 1536:2048])
            nc.sync.dma_start(wk[:, 4:8, :], wk_d[:, 4:8, :])
            for ko in range(4, 8):
                nc.sync.dma_start(
                    xts[0][:, ko, :], xt_d[0, :, ko * 512:(ko + 1) * 512])
            wv = const.tile([128, 8, GC], BF16)
            nc.sync.dma_start(wv[:], wv_d[:])
            tri = const.tile([128, 128], BF16)
            nc.sync.dma_start(tri[:], tri_d[:])
            for sb in (1, 2, 3):
                for qt in range(4):
                    nc.sync.dma_start(
                        xts[sb][:, 2 * qt:2 * qt + 2, :],
                        xt_d[sb, :, qt * 1024:(qt + 1) * 1024])
            wo = const.tile([128, 2, D], BF16)
            nc.sync.dma_start(wo[:], wo_d[:])

            # v tiles: per j-chunk, per head 64 value cols + 64 ones cols (the
            # ones columns make the PV matmul emit the softmax denominator in
            # PSUM rows 64:128)
            # ones in cols 0:64 (so the denominator lands at PSUM rows 0:64,
            # where the custom-DVE reciprocal can read it directly -- it
            # ignores partition offsets on HW), v values in cols 64:128.
            vts = []
            for io in range(16):
                vt = const.tile([128, HC, 128], BF16, name=f"v{io}")
                nc.vector.memset(vt[:, :, 0:64], 1.0)
                vts.append(vt)

            qts = [[const.tile([128, 512], BF16, name=f"q{m}{s}")
                    for s in range(4)] for m in range(2)]
            kts = [[const.tile([128, 512], BF16, name=f"k{m}{s}")
                    for s in range(4)] for m in range(2)]

            def proj_qk(sb):
                for w_sb, dst in ((wq, qts), (wk, kts)):
                    for mo in range(2):
                        ps = psmm.tile([128, 2, 512], F32, tag="mm")
                        for ko in range(8):
                            nc.tensor.matmul(
                                ps[:, 0, :],
                                w_sb[:, ko, mo * 128:(mo + 1) * 128],
                                xts[sb][:, ko, :],
                                start=(ko == 0),
                                stop=(ko == 7),
                                skip_group_check=True,
                            )
                        nc.scalar.copy(dst[mo][sb][:], ps[:, 0, :])

            def proj_v(io):
                sb, i2 = divmod(io, 4)
                ps = psout.tile([128, 512], F32, tag="out", name=f"vps{io}")
                for ko in range(8):
                    nc.tensor.matmul(
                        ps[:, 0:256],
                        xts[sb][:, ko, i2 * 128:(i2 + 1) * 128],
                        wv[:, ko, :],
                        start=(ko == 0),
                        stop=(ko == 7),
                        skip_group_check=True,
                    )
                nc.vector.tensor_copy(
                    vts[io][:, :, 64:128],
                    ps[:, 0:256].rearrange("p (h e) -> p h e", e=64),
                )

            # ---- attention + output projection ------------------------------
            def qk(Q, mo, jc):
                n_full = 4 * Q
                diag = jc >= n_full
                o = jc - n_full if diag else 0
                lo = o * 128 if diag else 0
                sc = psmm.tile([128, 2, 512], F32, tag="mm")
                for hp in range(2):
                    nc.tensor.matmul(
                        sc[:, hp, lo:512],
                        kts[mo][jc // 4][hp * 64:(hp + 1) * 64,
                                         (jc % 4) * 128:(jc % 4 + 1) * 128],
                        qts[mo][Q][hp * 64:(hp + 1) * 64, lo:512],
                        start=True,
                        stop=True,
                        skip_group_check=True,
                    )
                ex = work.tile([128, 2, 512], BF16, tag="exp")
                nc.scalar.activation(ex[:, :, lo:512], sc[:, :, lo:512],
                                     EXP, scale=SCALE)
                if diag:
                    for hp in range(2):
                        nc.vector.tensor_mul(
                            ex[:, hp, lo:lo + 128],
                            ex[:, hp, lo:lo + 128],
                            tri[:],
                        )
                return ex, lo

            def attn(Q, mo, pre=None, inserts=None):
                # pre: list of already-emitted qk() results (chunks 0..len-1),
                # used to warm the exp pipeline across the outproj junction
                n_full = 4 * Q
                nch = n_full + 4
                out_ps = [psout.tile([128, 512], F32, tag="out",
                                     name=f"ops{Q}{mo}{_h}") for _h in range(2)]

                def pv(jc, ex, lo):
                    for hp in range(2):
                        nc.tensor.matmul(
                            out_ps[hp][:, lo:512],
                            vts[jc][:, 2 * mo + hp, :],
                            ex[:, hp, lo:512],
                            start=(jc == 0),
                            stop=(jc == nch - 1),
                            skip_group_check=True,
                        )

                pend = list(pre) if pre else [qk(Q, mo, 0)]
                done = 0
                inserts = dict(inserts or {})
                for jc in range(len(pend), nch):
                    pend.append(qk(Q, mo, jc))
                    if jc in inserts:
                        proj_v(inserts.pop(jc))
                    pv(done, *pend[0])
                    pend.pop(0)
                    done += 1
                for p in pend:
                    pv(done, *p)
                    done += 1
                return out_ps

            def chains(out_ps, aTm, split=False):
                # normalization: rows 0:64 of out_ps hold the denominator
                # replicated 64x, rows 64:128 the numerator, so this is one
                # recip + one mul on DVE per head, done in column halves so
                # outproj's first aT slices are ready in half the latency.
                # Emitted separately from attn() so the DVE FIFO never parks
                # these behind the causal-mask muls a PV is waiting on.
                for cc in range(2 if split else 1):
                    cs = slice(cc * 256, (cc + 1) * 256) if split \
                        else slice(0, 512)
                    for hp in range(2):
                        rdb = rpool.tile([64, 512], F32, tag="rd")
                        nc.vector.reciprocal_approx_fast(
                            out=rdb[:, cs], in_=out_ps[hp][0:64, cs])
                        nc.vector.tensor_mul(
                            aTm[hp * 64:(hp + 1) * 64, cs],
                            out_ps[hp][64:128, cs],
                            rdb[:, cs],
                        )

            def outproj(Q, aTq):
                # so handled in pairs with all co=0 matmuls first, so the PE
                # has ~4 matmuls of runway before the first co=1 matmul needs
                # the (possibly still in flight) mo=1 normalization chain
                odr = out_d.rearrange("(a p) n -> p a n", p=128)
                for sp in range(2):
                    pos = []
                    for so2 in range(2):
                        po = psmm.tile([128, 2, 512], F32, tag="mm",
                                       name=f"po{sp}{so2}")
                        pos.append(po)
                    for co in range(2):
                        for so2 in range(2):
                            so = 2 * sp + so2
                            for nt in range(2):
                                nc.tensor.matmul(
                                    pos[so2][:, nt, :],
                                    aTq[co][:, so * 128:(so + 1) * 128],
                                    wo[:, co, nt * 512:(nt + 1) * 512],
                                    start=(co == 0),
                                    stop=(co == 1),
                                    skip_group_check=True,
                                )
                    for so2 in range(2):
                        so = 2 * sp + so2
                        osb = opool.tile([128, D], BF16, tag="osb")
                        nc.scalar.copy(osb[:, 0:512], pos[so2][:, 0, :])
                        nc.vector.tensor_copy(osb[:, 512:1024], pos[so2][:, 1, :])
                        for half in range(2):
                            nc.sync.dma_start(
                                odr[:, Q * 4 + so, half * 512:(half + 1) * 512],
                                osb[:, half * 512:(half + 1) * 512],
                            )

            def proj_qk0():
                ps_q = psmm.tile([128, 2, 512], F32, tag="mm")
                ps_k = psmm.tile([128, 2, 512], F32, tag="mm")
                steps = []
                for ko in range(8):
                    steps.append((wq, ps_q, ko))
                for ko in range(8):
                    steps.append((wk, ps_k, ko))
                order = []
                qi, ki = 0, 0
                for ko in range(8):
                    order.append(steps[ko])
                    if ko >= 2:
                        order.append(steps[8 + ko - 2])
                order += steps[8 + 6:]
                for w_sb, ps, ko in order:
                    for mo in range(2):
                        nc.tensor.matmul(
                            ps[:, mo, :],
                            w_sb[:, ko, mo * 128:(mo + 1) * 128],
                            xts[0][:, ko, :],
                            start=(ko == 0),
                            stop=(ko == 7),
                            skip_group_check=True,
                        )
                for ps, dst in ((ps_q, qts), (ps_k, kts)):
                    for mo in range(2):
                        nc.scalar.copy(dst[mo][0][:], ps[:, mo, :])

            aTs = []
            for Q in range(4):
                if Q == 0:
                    proj_qk0()
                else:
                    proj_qk(Q)
                if Q == 0:
                    for io in range(4):
                        proj_v(io)
                    ins0 = None
                else:
                    proj_v(4 * Q)
                    ins0 = {k: 4 * Q + k for k in (1, 2, 3)}
                aTq = [apool.tile([128, 512], BF16, tag="aT", name=f"aT{Q}{m}")
                       for m in range(2)]
                aTs.append(aTq)
                ops0 = attn(Q, 0, inserts=ins0)
                # chains(3,0) goes right before the big A(3,1) block so the
                # final outproj only waits on the short chains(3,1); for
                # other Q both chains sit after A(Q,1), clear of the DVE path
                # that feeds diag masks and osb copies.
                if Q == 3:
                    chains(ops0, aTq[0])
                pre = None
                if Q >= 1:
                    # warm A(Q,1)'s first two chunks across the outproj
                    # junction: their sc tiles take the mm slots ahead of po
                    # and their exps fill the pipeline during the po matmuls
                    pre = [qk(Q, 1, 0), qk(Q, 1, 1)]
                    outproj(Q - 1, aTs[Q - 1])
                ops1 = attn(Q, 1, pre=pre)
                if Q < 3:
                    chains(ops0, aTq[0])
                chains(ops1, aTq[1], split=(Q == 3))
            outproj(3, aTs[3])

    nc.compile()
    return nc


_NC = None


def _get_nc():
    global _NC
    if _NC is None:
        _NC = build_bass()
    return _NC


def _prep_core_inputs(x, Wq, Wk, Wv, Wo, c):
    b, g = divmod(c, 4)
    cols = slice(g * GC, (g + 1) * GC)
    xT = np.ascontiguousarray(x[b].T).astype(NPBF16)          # [1024, 2048]
    # xt[sb][p, ko*512+s] = xT[ko*128+p, sb*512+s]
    xt = np.ascontiguousarray(
        xT.reshape(8, 128, 4, 512).transpose(2, 1, 0, 3).reshape(4, 128, 8 * 512))

    def wtile(W):  # W.T[:, cols] -> [128, 8, 256]
        wt = np.ascontiguousarray(W.T[:, cols]).astype(NPBF16)
        return np.ascontiguousarray(wt.reshape(8, 128, GC).transpose(1, 0, 2))

    woT = np.ascontiguousarray(Wo[:, cols].T).astype(NPBF16)  # [256, 1024]
    wo = np.ascontiguousarray(woT.reshape(2, 128, D).transpose(1, 0, 2))
    tri = (np.arange(128)[:, None] <= np.arange(128)[None, :]).astype(NPBF16)
    return {
        "xt": xt,
        "wq": wtile(Wq),
        "wk": wtile(Wk),
        "wv": wtile(Wv),
        "wo": wo,
        "tri": tri,
    }


def kernel(in_features, Wq, Wk, Wv, Wo):
    global LAST_RESULTS
    nc = _get_nc()

    x = np.asarray(in_features, np.float32)
    Wq = np.asarray(Wq, np.float32)
    Wk = np.asarray(Wk, np.float32)
    Wv = np.asarray(Wv, np.float32)
    Wo = np.asarray(Wo, np.float32)

    in_maps = [_prep_core_inputs(x, Wq, Wk, Wv, Wo, c) for c in range(N_CORES)]

    res = bass_utils.run_bass_kernel_spmd(
        nc, in_maps, core_ids=list(range(N_CORES)), trace=TRACE,
    )
    LAST_RESULTS = res
    parts = [np.asarray(res.results[c]["out"], np.float32)
             for c in range(N_CORES)]
    out = np.stack([
        parts[4 * b] + parts[4 * b + 1] + parts[4 * b + 2] + parts[4 * b + 3]
        for b in range(B)
    ]).astype(np.float32)
    return out
